# revision 23
# baseline (speedup 1.0000x reference)
"""Fused transformer-block kernel for TRN2, 8-way data parallel over batch.

v2: fp8e4 DoubleRow matmuls for all large GEMMs (QK, V, proj, MLP1, MLP2),
~1.4x PE throughput over the bf16 baseline.  Weights are staged as
fp8(32*w); every 1/32 descale folds into an existing free scale slot
(softmax exp scale, softmax reciprocal, gelu scale, residual STTs).

Other changes vs v1:
  - x/y move over DRAM as bf16 (half the residual DMA traffic).
  - LN stats via one-pass bn_stats/bn_aggr; rstd via a Quake-style
    bit-trick rsqrt + 1 Newton step on the DVE (no ACT Sqrt, so the ACT
    table only flips between exp_and_others and gelu_and_others).
  - Causal mask folded into the scores PSUM via an identity-weight
    matmul preloading -1e6 into the masked triangle (no DVE mask mult).
    Each PSUM region's accumulation group (mask preload + QK matmul)
    must stay consecutive - interleaved groups miscompute on HW.
  - All transposes on the PE (the DMA xbar at ~26 GB/s per ring was
    both slower and a serialization hazard).
  - Blocks processed in pairs so exp/gelu table switches amortize over
    two blocks; gelu batched over 2-fm PSUM groups (N=768 per ACT op).
  - Emission order per iteration puts latency-tolerant work (LN1 chain)
    at the per-engine FIFO tails: consumers wait on producer-engine
    instruction COUNTERS, so anything emitted ahead of a producer delays
    its consumers transitively.
"""

import sys

sys.path.insert(0, "/opt/trn_rl_repo")

from contextlib import ExitStack

import ml_dtypes
import numpy as np

import concourse.bass as bass  # noqa: F401  (registers AP types)
import concourse.tile as tile
from concourse import bacc, bass_utils, mybir

# Cache walrus-compiled NEFFs on disk keyed by BIR hash.
try:
    import hashlib
    import os as _os
    import shutil as _shutil

    import concourse.bass2jax as _b2j

    _orig_cbk = _b2j.compile_bir_kernel

    def _cached_cbk(bir_json, tmpdir, neff_name="file.neff"):
        try:
            raw = bir_json if isinstance(bir_json, bytes) else bir_json.encode()
            h = hashlib.sha256(raw).hexdigest()[:24]
            cdir = "/tmp/neff_cache"
            _os.makedirs(cdir, exist_ok=True)
            cpath = _os.path.join(cdir, h + ".neff")
            if _os.path.exists(cpath):
                return cpath
        except Exception:
            return _orig_cbk(bir_json, tmpdir, neff_name)
        p = _orig_cbk(bir_json, tmpdir, neff_name)
        try:
            _shutil.copy(p, cpath)
        except Exception:
            pass
        return p

    if _orig_cbk.__name__ != "_cached_cbk":
        _b2j.compile_bir_kernel = _cached_cbk
except Exception:
    pass

B, T, C = 1024, 96, 512
H, D = 4, 128
F = 4 * C
EPS = 1e-5
SCALE = D**-0.5

NCORES = 8
SEQ_PER_CORE = B // NCORES  # 128
S = SEQ_PER_CORE * T  # 12288 tokens per core
NB = 4  # sequences per block
TOK = NB * T  # 384 tokens per block
NBLK = SEQ_PER_CORE // NB  # 32 blocks
TCH = TOK // 128  # 3 token chunks per block
KC = C // 128  # 4 feature chunks of C
FM = F // 128  # 16 feature chunks of F

WS = 32.0  # weight staging scale (power of two)
IS = 1.0 / WS

F32 = mybir.dt.float32
BF16 = mybir.dt.bfloat16
F8 = mybir.dt.float8e4
I32 = mybir.dt.int32
AF = mybir.ActivationFunctionType
OP = mybir.AluOpType
DR = mybir.MatmulPerfMode.DoubleRow

QMAGIC = 0x5F3759DF


def build(nblk=NBLK, has_bq=False, has_bk=False, has_bv=False, has_bp=False,
          has_b1=False, has_b2=False):
    nc = bacc.Bacc("TRN2", target_bir_lowering=False, debug=False)

    def din(name, shape, dt):
        return nc.dram_tensor(name, shape, dt, kind="ExternalInput").ap()

    x_d = din("x", [S, C], BF16)
    wq_d = din("wq", [C, C], F8)
    wk_d = din("wk", [C, C], F8)
    wv_d = din("wv", [C, C], F8)
    wp_d = din("wp", [C, C], F8)
    w1_d = din("w1", [C, F], F8)
    w2_d = din("w2", [F, C], F8)
    b1_d = din("b1", [F], F32) if has_b1 else None
    maskadd_d = din("maskadd", [T, T], BF16)
    ident_d = din("ident", [128, 128], BF16)
    bq_d = din("bq", [C], F32) if has_bq else None
    bk_d = din("bk", [C], F32) if has_bk else None
    bv_d = din("bv_b", [T, C], F32) if has_bv else None
    bp_d = din("bp_b", [128, C], F32) if has_bp else None
    b2_d = din("b2_b", [128, C], F32) if has_b2 else None
    y_d = nc.dram_tensor("y", [S, C], F32, kind="ExternalOutput").ap()

    # exp scale: softmax SCALE plus both q and k carrying x32 weights
    exp_scale = SCALE * IS * IS
    if has_bq or has_bk:
        exp_scale = SCALE  # descale applied at the qt/kt copies instead

    with tile.TileContext(nc) as tc, ExitStack() as ctx:
        wp = ctx.enter_context(tc.tile_pool(name="wpool", bufs=1))
        ap_ = ctx.enter_context(tc.tile_pool(name="act", bufs=2))
        st = ctx.enter_context(tc.tile_pool(name="stat", bufs=3))
        hp = ctx.enter_context(tc.tile_pool(name="ht", bufs=1))
        ps = ctx.enter_context(tc.tile_pool(name="psum", bufs=1, space="PSUM"))

        # ---- resident weights ----
        def wload(name, d_ap, kchunks, fdim, dt, eng=None):
            t = wp.tile([128, kchunks, fdim], dt, tag=name)
            (eng or nc.sync).dma_start(
                t[:], d_ap.rearrange("(kc p) f -> p kc f", p=128))
            return t

        # small constants first, then weights in first-use order, all on
        # the sync ring (x/y traffic lives on gpsimd)
        maskadd_sb = wp.tile([T, T], BF16, tag="maskadd")
        nc.sync.dma_start(maskadd_sb[:], maskadd_d)
        ident_sb = wp.tile([128, 128], BF16, tag="ident")
        nc.sync.dma_start(ident_sb[:], ident_d)
        wq_sb = wload("wq", wq_d, KC, C, F8, nc.sync)
        wk_sb = wload("wk", wk_d, KC, C, F8, nc.sync)
        wv_sb = wload("wv", wv_d, KC, C, F8, nc.sync)
        wp_sb = wload("wp", wp_d, KC, C, F8, nc.sync)
        w1_sb = wload("w1", w1_d, KC, F, F8, nc.sync)
        w2_sb = wload("w2", w2_d, FM, C, F8, nc.sync)

        if has_b1:
            b1_sb = wp.tile([128, FM], F32, tag="b1")
            nc.sync.dma_start(b1_sb[:], b1_d.rearrange("(fm p) -> p fm", p=128))
        if has_bq:
            bq_sb = wp.tile([128, H], F32, tag="bq")
            nc.sync.dma_start(bq_sb[:], bq_d.rearrange("(h d) -> d h", d=128))
        if has_bk:
            bk_sb = wp.tile([128, H], F32, tag="bk")
            nc.sync.dma_start(bk_sb[:], bk_d.rearrange("(h d) -> d h", d=128))
        if has_bv:
            bv_sb = wp.tile([T, C], F32, tag="bv")
            nc.sync.dma_start(bv_sb[:], bv_d)
        if has_bp:
            bp_sb = wp.tile([128, C], F32, tag="bp")
            nc.sync.dma_start(bp_sb[:], bp_d)
        if has_b2:
            b2_sb = wp.tile([128, C], F32, tag="b2")
            nc.sync.dma_start(b2_sb[:], b2_d)

        # ---- LN helpers ----
        # Stats are collected per block into a shared per-pair agg tile;
        # the rsqrt (quake bit trick + 1 Newton step, all DVE, no ACT
        # table) runs ONCE per pair over both blocks' stats so its serial
        # chain amortizes.
        def ln_stats_collect(src, agg, j, pref):
            bst = st.tile([128, TCH, 6], F32, tag=pref + "bst")
            for i in range(TCH):
                nc.vector.bn_stats(bst[:, i, :], src[:, i, :])
                nc.vector.bn_aggr(agg[:, j, i, :], bst[:, i, :])

        def quake_pair(agg, pref):
            """agg [128, 2, TCH, 2] -> (rstd, nmr) [128, 2, TCH] f32."""
            mu = agg[:, :, :, 0]
            var = agg[:, :, :, 1]
            veps = st.tile([128, 2, TCH], F32, tag=pref + "veps")
            nc.vector.tensor_scalar_add(veps[:], var, EPS)
            ti = st.tile([128, 2, TCH], I32, tag=pref + "ti")
            nc.vector.tensor_scalar(
                out=ti[:], in0=veps[:].bitcast(I32), scalar1=1, scalar2=None,
                op0=OP.arith_shift_right)
            # magic - t == (t XOR -1) + (magic + 1)  (two's complement;
            # bitwise and arith ops can't share one instruction)
            nc.vector.tensor_scalar(
                out=ti[:], in0=ti[:], scalar1=-1, scalar2=None,
                op0=OP.bitwise_xor)
            nc.vector.tensor_scalar(
                out=ti[:], in0=ti[:], scalar1=QMAGIC + 1, scalar2=None,
                op0=OP.add)
            y0 = ti[:].bitcast(F32)
            a = st.tile([128, 2, TCH], F32, tag=pref + "nwa")
            nc.vector.tensor_mul(out=a[:], in0=y0, in1=y0)
            nc.vector.tensor_mul(out=a[:], in0=a[:], in1=veps[:])
            b = st.tile([128, 2, TCH], F32, tag=pref + "nwb")
            nc.vector.scalar_tensor_tensor(b[:], a[:], -0.5, y0,
                                           OP.mult, OP.mult)
            rstd = st.tile([128, 2, TCH], F32, tag=pref + "rstd")
            nc.vector.scalar_tensor_tensor(rstd[:], y0, 1.5, b[:],
                                           OP.mult, OP.add)
            nmr = st.tile([128, 2, TCH], F32, tag=pref + "nmr")
            nc.vector.scalar_tensor_tensor(nmr[:], mu, -1.0, rstd[:],
                                           OP.mult, OP.mult)
            return rstd, nmr

        def ln_apply(src, rstd, nmr, j, pref):
            xn = ap_.tile([128, TCH, C], BF16, tag=pref + "xn", bufs=4)
            for i in range(TCH):
                nc.scalar.activation(xn[:, i, :], src[:, i, :], AF.Identity,
                                     scale=rstd[:, j, i : i + 1],
                                     bias=nmr[:, j, i : i + 1])
            return xn

        # ---- block stages ----
        def a1_pair(pr):
            """x loads, LN1 (pair-shared quake), PE transposes, fp8 cast."""
            out = []
            blks = (2 * pr, 2 * pr + 1)
            xsb = {}
            for blk in blks:
                row0 = blk * TOK
                x_sb = ap_.tile([128, TCH, C], BF16, tag="x", bufs=8)
                nc.gpsimd.dma_start(
                    x_sb[:],
                    x_d[row0 : row0 + TOK, :].rearrange("(ch p) c -> p ch c",
                                                        p=128))
                xsb[blk] = x_sb
            agg = st.tile([128, 2, TCH, 2], F32, tag="aagg")
            for j, blk in enumerate(blks):
                ln_stats_collect(xsb[blk], agg, j, "a")
            rstd, nmr = quake_pair(agg, "a")
            for j, blk in enumerate(blks):
                xn = ln_apply(xsb[blk], rstd, nmr, j, "a")
                xnT = ap_.tile([128, KC, TOK], BF16, tag="axnT")
                # LN1 transposes ride the otherwise-idle DMA xbar (~30us
                # per pair on a ~59us pair cadence); the chain has two
                # iterations of slack.  LN2 stays on the PE - both LNs on
                # the xbar would saturate the ring.  NOTE: all xbar
                # transposes must stay on ONE HWDGE ring (sync).
                for kc in range(KC):
                    for mc in range(TCH):
                        nc.sync.dma_start_transpose(
                            out=xnT[:, kc, mc * 128 : (mc + 1) * 128],
                            in_=xn[:, mc, kc * 128 : (kc + 1) * 128])
                xnT8 = ap_.tile([128, KC, TOK], F8, tag="axnT8", bufs=6)
                nc.vector.tensor_copy(out=xnT8[:], in_=xnT[:])
                out.append((xsb[blk], xnT8))
            return out

        def stage_a2(blk, xnT8):
            """QKV (fp8 DoubleRow) + masked scores + exp + softmax denom."""
            qt = ap_.tile([128, H, TOK], BF16, tag="qt")
            kt = ap_.tile([128, H, TOK], BF16, tag="kt")
            for dst, w_sb, bias_sb in ((qt, wq_sb, bq_sb if has_bq else None),
                                       (kt, wk_sb, bk_sb if has_bk else None)):
                for h in range(H):
                    p = ps.tile([128, 512], F32, tag="pa", bufs=4)
                    for kc in range(0, KC, 2):
                        nc.tensor.matmul(
                            p[:, 0:TOK],
                            w_sb[:, kc : kc + 2, h * 128 : (h + 1) * 128],
                            xnT8[:, kc : kc + 2, :],
                            start=(kc == 0), stop=(kc == KC - 2), perf_mode=DR)
                    if bias_sb is not None:
                        nc.scalar.activation(dst[:, h, :], p[:, 0:TOK],
                                             AF.Identity, scale=IS,
                                             bias=bias_sb[:, h : h + 1])
                    elif has_bq or has_bk:
                        # the IS^2 fold into exp_scale is off in bias mode
                        nc.scalar.activation(dst[:, h, :], p[:, 0:TOK],
                                             AF.Identity, scale=IS)
                    elif h % 2 == 0:
                        nc.scalar.activation(dst[:, h, :], p[:, 0:TOK],
                                             AF.Identity)
                    else:
                        nc.vector.tensor_copy(out=dst[:, h, :], in_=p[:, 0:TOK])

            # scores with additive causal mask preloaded into PSUM
            ee = ap_.tile([T, H * NB, T], BF16, tag="ee", bufs=4)
            dsum = st.tile([T, H * NB], F32, tag="dsum")
            for h in range(H):
                p = ps.tile([T, NB, T], F32, tag="pa", bufs=4)
                # keep each region's accumulation group consecutive: the
                # interleaved form (all starts, then all stops) computes
                # wrong results on hardware.
                for b in range(NB):
                    nc.tensor.matmul(p[:, b, :], ident_sb[:T, :T],
                                     maskadd_sb[:], start=True, stop=False)
                    nc.tensor.matmul(p[:, b, :], qt[:, h, b * T : (b + 1) * T],
                                     kt[:, h, b * T : (b + 1) * T],
                                     start=False, stop=True)
                sl = slice(h * NB, (h + 1) * NB)
                nc.scalar.activation(ee[:, sl, :], p[:], AF.Exp,
                                     scale=exp_scale)
            nc.vector.tensor_reduce(dsum[:], ee[:], axis=mybir.AxisListType.X,
                                    op=OP.add)
            # rr = IS / dsum  (folds away the x32 on wv; with bv the
            # descale already happened at the vt copy)
            rr = st.tile([T, H * NB], F32, tag="rr")
            if has_bv:
                nc.vector.reciprocal(rr[:], dsum[:])
            else:
                nc.vector.tensor_scalar_mul(rr[:], dsum[:], WS)
                nc.vector.reciprocal(rr[:], rr[:])
            nc.vector.tensor_mul(
                out=ee[:], in0=ee[:],
                in1=rr[:].unsqueeze(2).to_broadcast([T, H * NB, T]))

            # V projection (fp8 DoubleRow, per sequence)
            vt = ap_.tile([T, NB, C], BF16, tag="vt", bufs=4)
            for b in range(NB):
                p = ps.tile([T, 512], F32, tag="pa", bufs=4)
                for kc in range(0, KC, 2):
                    nc.tensor.matmul(p[:], xnT8[:, kc : kc + 2, b * T : (b + 1) * T],
                                     wv_sb[:, kc : kc + 2, :],
                                     start=(kc == 0), stop=(kc == KC - 2),
                                     perf_mode=DR)
                if has_bv:
                    nc.vector.scalar_tensor_tensor(vt[:, b, :], p[:], IS,
                                                   bv_sb[:], OP.mult, OP.add)
                elif b % 2 == 0:
                    nc.scalar.activation(vt[:, b, :], p[:], AF.Identity)
                else:
                    nc.vector.tensor_copy(out=vt[:, b, :], in_=p[:])
            return vt, ee

        def stage_a2b(blk, vt, ee):
            """probs transpose + attn @ V -> ot8 (fp8, T-layout)."""
            pt = ee  # probs overwritten in place by their transpose
            for h in range(H):
                p = ps.tile([T, NB, T], BF16, tag="pa", bufs=4)
                for b in range(NB):
                    nc.tensor.transpose(p[:, b, :], ee[:, h * NB + b, :],
                                        ident_sb[:T, :T])
                nc.vector.tensor_copy(out=pt[:, h * NB : (h + 1) * NB, :], in_=p[:])
            ot8 = ap_.tile([128, H, TOK], F8, tag="ot8", bufs=4)
            for h in range(H):
                p = ps.tile([128, NB, T], F32, tag="pa", bufs=4)
                for b in range(NB):
                    nc.tensor.matmul(p[:, b, :], vt[:, b, h * 128 : (h + 1) * 128],
                                     pt[:, h * NB + b, :], start=True, stop=True)
                if h % 2 == 0:
                    nc.scalar.activation(ot8[:, h, :], p[:], AF.Identity)
                else:
                    nc.vector.tensor_copy(out=ot8[:, h, :], in_=p[:])
            return ot8

        def stage_b1(blk, x_sb, ot8, agg2, j):
            """proj + residual -> x2; LN2 stats into the pair agg tile."""
            x2 = ap_.tile([128, TCH, C], BF16, tag="x2", bufs=4)
            for mc in range(TCH):
                p = ps.tile([128, 512], F32, tag="pb", bufs=2)
                for kc in range(0, H, 2):
                    nc.tensor.matmul(p[:], ot8[:, kc : kc + 2, mc * 128 : (mc + 1) * 128],
                                     wp_sb[:, kc : kc + 2, :],
                                     start=(kc == 0), stop=(kc == H - 2),
                                     perf_mode=DR)
                if has_bp:
                    nc.vector.scalar_tensor_tensor(p[:], p[:], IS, bp_sb[:],
                                                   OP.mult, OP.add)
                    nc.vector.tensor_add(out=x2[:, mc, :], in0=p[:],
                                         in1=x_sb[:, mc, :])
                else:
                    # x2 = proj * IS + x
                    nc.vector.scalar_tensor_tensor(
                        x2[:, mc, :], p[:], IS, x_sb[:, mc, :], OP.mult, OP.add)
            ln_stats_collect(x2, agg2, j, "b")
            return x2

        def stage_b1b(blk, x2, rstd2, nmr2, j):
            """LN2 apply (ACT) -> xn2."""
            return ln_apply(x2, rstd2, nmr2, j, "b")

        def stage_b2(blk, xn2):
            """LN2 transpose on the PE + one batched fp8 cast."""
            tp = ps.tile([128, KC * TCH, 128], BF16, tag="pb", bufs=2)
            for kc in range(KC):
                for mc in range(TCH):
                    nc.tensor.transpose(tp[:, kc * TCH + mc, :],
                                        xn2[:, mc, kc * 128 : (kc + 1) * 128],
                                        ident_sb[:])
            xn2T8 = ap_.tile([128, KC, TOK], F8, tag="xn2T8")
            nc.vector.tensor_copy(out=xn2T8[:], in_=tp[:])
            return xn2T8

        def stage_b3(blk, x2, xn2T8):
            """MLP (fp8 DoubleRow) + residual + store."""
            row0 = blk * TOK
            ht = hp.tile([128, FM, TOK], F8, tag="ht")
            for fg in range(0, FM, 2):
                p = ps.tile([128, 2, 512], F32, tag="pb", bufs=2)
                for fi in range(2):
                    fm = fg + fi
                    for kc in range(0, KC, 2):
                        nc.tensor.matmul(
                            p[:, fi, 0:TOK],
                            w1_sb[:, kc : kc + 2, fm * 128 : (fm + 1) * 128],
                            xn2T8[:, kc : kc + 2, :],
                            start=(kc == 0), stop=(kc == KC - 2), perf_mode=DR)
                if has_b1:
                    for fi in range(2):
                        nc.scalar.activation(ht[:, fg + fi, :], p[:, fi, 0:TOK],
                                             AF.Gelu, scale=IS,
                                             bias=b1_sb[:, fg + fi : fg + fi + 1])
                else:
                    nc.scalar.activation(ht[:, fg : fg + 2, :], p[:, :, 0:TOK],
                                         AF.Gelu, scale=IS)

            xo = ap_.tile([128, TCH, C], F32, tag="xo")
            for mc in range(TCH):
                p = ps.tile([128, 512], F32, tag="pb", bufs=2)
                for fk in range(0, FM, 2):
                    nc.tensor.matmul(p[:], ht[:, fk : fk + 2, mc * 128 : (mc + 1) * 128],
                                     w2_sb[:, fk : fk + 2, :],
                                     start=(fk == 0), stop=(fk == FM - 2),
                                     perf_mode=DR)
                if has_b2:
                    nc.vector.scalar_tensor_tensor(p[:], p[:], IS, b2_sb[:],
                                                   OP.mult, OP.add)
                    nc.vector.tensor_add(out=xo[:, mc, :], in0=p[:],
                                         in1=x2[:, mc, :])
                else:
                    nc.vector.scalar_tensor_tensor(
                        xo[:, mc, :], p[:], IS, x2[:, mc, :], OP.mult, OP.add)
            nc.gpsimd.dma_start(
                y_d[row0 : row0 + TOK, :].rearrange("(ch p) c -> p ch c", p=128),
                xo[:])

        # ---- software-pipelined emission over block PAIRS ----
        # Per iteration (pair pr; prev pair P1,P2):
        #   a1_pair(pr+2)   LN1 chain, two pairs of latency slack
        #   b2(P1) b2(P2)   prev pair LN2 transposes (applies done last iter)
        #   a2(A) a2(B)     attention front (PE-heavy, hides LN2+softmax
        #   b3(P1) b3(P2)   prev pair MLPs    chains queued further down)
        #   a2b(A) a2b(B)   attention tail
        #   b1 b1 quake b1b b1b   proj + LN2 chain; latency hidden until
        #                         the NEXT iteration's b2 needs xn2
        # The ACT table flips exp_and_others <-> gelu_and_others only twice
        # per pair (all Identity/LN applies live in every set).
        assert nblk % 2 == 0
        npair = nblk // 2
        xs, xnTs, sm, ots, x2s, xn2s = {}, {}, {}, {}, {}, {}

        def do_a1(pr):
            for blk, (x_sb, xnT8) in zip((2 * pr, 2 * pr + 1), a1_pair(pr)):
                xs[blk], xnTs[blk] = x_sb, xnT8

        def do_b1(pr):
            agg2 = st.tile([128, 2, TCH, 2], F32, tag="bagg")
            for j, blk in enumerate((2 * pr, 2 * pr + 1)):
                x2s[blk] = stage_b1(blk, xs.pop(blk), ots.pop(blk), agg2, j)
            rstd2, nmr2 = quake_pair(agg2, "b")
            for j, blk in enumerate((2 * pr, 2 * pr + 1)):
                xn2s[blk] = stage_b1b(blk, x2s[blk], rstd2, nmr2, j)

        do_a1(0)
        if npair > 1:
            do_a1(1)
        for pr in range(npair):
            if pr > 0:
                for blk in (2 * pr - 2, 2 * pr - 1):
                    xn2s[blk] = stage_b2(blk, xn2s.pop(blk))
            for blk in (2 * pr, 2 * pr + 1):
                sm[blk] = stage_a2(blk, xnTs.pop(blk))
            if pr > 0:
                for blk in (2 * pr - 2, 2 * pr - 1):
                    stage_b3(blk, x2s.pop(blk), xn2s.pop(blk))
            for blk in (2 * pr, 2 * pr + 1):
                ots[blk] = stage_a2b(blk, *sm.pop(blk))
            do_b1(pr)
            # a1 is latency-tolerant (consumed two iterations later), so
            # its DVE/ACT/PE work goes at the queue TAILS: the in-order
            # engine FIFOs then serve this iteration's critical items
            # (casts, qt/kt copies, softmax) first.
            if pr + 2 < npair:
                do_a1(pr + 2)
        for blk in (nblk - 2, nblk - 1):
            xn2s[blk] = stage_b2(blk, xn2s.pop(blk))
        for blk in (nblk - 2, nblk - 1):
            stage_b3(blk, x2s.pop(blk), xn2s.pop(blk))

    nc.compile()
    return nc


def fold(inputs):
    """Host-side exact folding of LN affines and biases into weights."""
    f32 = np.float32
    f8 = ml_dtypes.float8_e4m3
    g1 = np.asarray(inputs["g1"], f32)
    be1 = np.asarray(inputs["be1"], f32)
    g2 = np.asarray(inputs["g2"], f32)
    be2 = np.asarray(inputs["be2"], f32)

    def headcat(w):  # [H, C, D] -> [C, H*D]
        return np.concatenate([w[h] for h in range(H)], axis=1)

    wq = headcat(np.asarray(inputs["wq"], f32))
    wk = headcat(np.asarray(inputs["wk"], f32))
    wv = headcat(np.asarray(inputs["wv"], f32))
    wp_ = np.asarray(inputs["w_proj"], f32)
    w1 = np.asarray(inputs["w1"], f32)
    w2 = np.asarray(inputs["w2"], f32)

    wq_f = g1[:, None] * wq
    wk_f = g1[:, None] * wk
    wv_f = g1[:, None] * wv
    bq = be1 @ wq
    bk = be1 @ wk
    bv = be1 @ wv
    bp = np.asarray(inputs["b_proj"], f32)
    w1_f = g2[:, None] * w1
    b1 = np.asarray(inputs["b1"], f32) + be2 @ w1
    b2 = np.asarray(inputs["b2"], f32)

    def q8(w):
        return np.clip(w * WS, -240.0, 240.0).astype(f8)

    # Masked positions must vanish under exp(exp_scale * psum); the psum
    # carries q.k at x1024 (both operands hold x32 weights), so the
    # additive mask must be large enough that exp_scale*mask << -40.
    maskadd = np.where(np.tril(np.ones((T, T), np.float32)) > 0, 0.0,
                       -1e6).astype(ml_dtypes.bfloat16)
    ident = np.eye(128, dtype=ml_dtypes.bfloat16)

    staged = {
        "wq": q8(wq_f),
        "wk": q8(wk_f),
        "wv": q8(wv_f),
        "wp": q8(wp_),
        "w1": q8(w1_f),
        "w2": q8(w2),
        "maskadd": maskadd,
        "ident": ident,
    }
    flags = {
        "has_bq": bool(np.any(bq)),
        "has_bk": bool(np.any(bk)),
        "has_bv": bool(np.any(bv)),
        "has_bp": bool(np.any(bp)),
        "has_b1": bool(np.any(b1)),
        "has_b2": bool(np.any(b2)),
    }
    if flags["has_b1"]:
        staged["b1"] = b1
    if flags["has_bq"]:
        staged["bq"] = bq
    if flags["has_bk"]:
        staged["bk"] = bk
    if flags["has_bv"]:
        staged["bv_b"] = np.broadcast_to(bv, (T, C)).copy()
    if flags["has_bp"]:
        staged["bp_b"] = np.broadcast_to(bp, (128, C)).copy()
    if flags["has_b2"]:
        staged["b2_b"] = np.broadcast_to(b2, (128, C)).copy()
    return staged, flags


_CACHE = {}


def kernel(**inputs):
    inputs = {k: np.asarray(v) for k, v in inputs.items()}
    staged, flags = fold(inputs)
    key = tuple(sorted(flags.items()))
    if key not in _CACHE:
        _CACHE[key] = build(**flags)
    nc = _CACHE[key]

    x = np.asarray(inputs["x"], np.float32).astype(ml_dtypes.bfloat16)
    x = x.reshape(B, T * C)
    in_maps = []
    for c in range(NCORES):
        m = dict(staged)
        m["x"] = x[c * SEQ_PER_CORE : (c + 1) * SEQ_PER_CORE].reshape(S, C)
        in_maps.append(m)

    res = bass_utils.run_bass_kernel_spmd(nc, in_maps, core_ids=list(range(NCORES)))
    out = np.concatenate([r["y"] for r in res.results], axis=0)
    return out.reshape(B, T, C).astype(np.float32)


# revision 24
# speedup vs baseline: 1.5772x; 1.5772x over previous
"""Fused transformer-block kernel for TRN2, 8-way data parallel over batch.

v2: fp8e4 DoubleRow matmuls for all large GEMMs (QK, V, proj, MLP1, MLP2),
~1.4x PE throughput over the bf16 baseline.  Weights are staged as
fp8(32*w); every 1/32 descale folds into an existing free scale slot
(softmax exp scale, softmax reciprocal, gelu scale, residual STTs).

Other changes vs v1:
  - x/y move over DRAM as bf16 (half the residual DMA traffic).
  - LN stats via one-pass bn_stats/bn_aggr; rstd via a Quake-style
    bit-trick rsqrt + 1 Newton step on the DVE (no ACT Sqrt, so the ACT
    table only flips between exp_and_others and gelu_and_others).
  - Causal mask folded into the scores PSUM via an identity-weight
    matmul preloading -1e4 into the masked triangle (no DVE mask mult).
  - Blocks processed in pairs so exp/gelu table switches amortize over
    two blocks; gelu batched over 2-fm PSUM groups (N=768 per ACT op).
"""

import sys

sys.path.insert(0, "/opt/trn_rl_repo")

from contextlib import ExitStack

import ml_dtypes
import numpy as np

import concourse.bass as bass  # noqa: F401  (registers AP types)
import concourse.tile as tile
from concourse import bacc, bass_utils, mybir

# Cache walrus-compiled NEFFs on disk keyed by BIR hash.
try:
    import hashlib
    import os as _os
    import shutil as _shutil

    import concourse.bass2jax as _b2j

    _orig_cbk = _b2j.compile_bir_kernel

    def _cached_cbk(bir_json, tmpdir, neff_name="file.neff"):
        try:
            raw = bir_json if isinstance(bir_json, bytes) else bir_json.encode()
            h = hashlib.sha256(raw).hexdigest()[:24]
            cdir = "/tmp/neff_cache"
            _os.makedirs(cdir, exist_ok=True)
            cpath = _os.path.join(cdir, h + ".neff")
            if _os.path.exists(cpath):
                return cpath
        except Exception:
            return _orig_cbk(bir_json, tmpdir, neff_name)
        p = _orig_cbk(bir_json, tmpdir, neff_name)
        try:
            _shutil.copy(p, cpath)
        except Exception:
            pass
        return p

    if _orig_cbk.__name__ != "_cached_cbk":
        _b2j.compile_bir_kernel = _cached_cbk
except Exception:
    pass

B, T, C = 1024, 96, 512
H, D = 4, 128
F = 4 * C
EPS = 1e-5
SCALE = D**-0.5

NCORES = 8
SEQ_PER_CORE = B // NCORES  # 128
S = SEQ_PER_CORE * T  # 12288 tokens per core
NB = 4  # sequences per block
TOK = NB * T  # 384 tokens per block
NBLK = SEQ_PER_CORE // NB  # 32 blocks
TCH = TOK // 128  # 3 token chunks per block
KC = C // 128  # 4 feature chunks of C
FM = F // 128  # 16 feature chunks of F

WS = 32.0  # weight staging scale (power of two)
IS = 1.0 / WS

F32 = mybir.dt.float32
BF16 = mybir.dt.bfloat16
F8 = mybir.dt.float8e4
I32 = mybir.dt.int32
AF = mybir.ActivationFunctionType
OP = mybir.AluOpType
DR = mybir.MatmulPerfMode.DoubleRow

QMAGIC = 0x5F3759DF


def build(nblk=NBLK, has_bq=False, has_bk=False, has_bv=False, has_bp=False,
          has_b1=False, has_b2=False):
    nc = bacc.Bacc("TRN2", target_bir_lowering=False, debug=False)

    def din(name, shape, dt):
        return nc.dram_tensor(name, shape, dt, kind="ExternalInput").ap()

    x_d = din("x", [S, C], BF16)
    wq_d = din("wq", [C, C], F8)
    wk_d = din("wk", [C, C], F8)
    wv_d = din("wv", [C, C], F8)
    wp_d = din("wp", [C, C], F8)
    w1_d = din("w1", [C, F], F8)
    w2_d = din("w2", [F, C], F8)
    b1_d = din("b1", [F], F32) if has_b1 else None
    maskadd_d = din("maskadd", [T, T], BF16)
    ident_d = din("ident", [128, 128], BF16)
    bq_d = din("bq", [C], F32) if has_bq else None
    bk_d = din("bk", [C], F32) if has_bk else None
    bv_d = din("bv_b", [T, C], F32) if has_bv else None
    bp_d = din("bp_b", [128, C], F32) if has_bp else None
    b2_d = din("b2_b", [128, C], F32) if has_b2 else None
    y_d = nc.dram_tensor("y", [S, C], F32, kind="ExternalOutput").ap()

    # exp scale: softmax SCALE plus both q and k carrying x32 weights
    exp_scale = SCALE * IS * IS
    if has_bq or has_bk:
        exp_scale = SCALE  # descale applied at the qt/kt copies instead

    with tile.TileContext(nc) as tc, ExitStack() as ctx:
        wp = ctx.enter_context(tc.tile_pool(name="wpool", bufs=1))
        ap_ = ctx.enter_context(tc.tile_pool(name="act", bufs=2))
        st = ctx.enter_context(tc.tile_pool(name="stat", bufs=3))
        hp = ctx.enter_context(tc.tile_pool(name="ht", bufs=1))
        ps = ctx.enter_context(tc.tile_pool(name="psum", bufs=1, space="PSUM"))

        # ---- resident weights ----
        def wload(name, d_ap, kchunks, fdim, dt, eng=None):
            t = wp.tile([128, kchunks, fdim], dt, tag=name)
            (eng or nc.sync).dma_start(
                t[:], d_ap.rearrange("(kc p) f -> p kc f", p=128))
            return t

        # small constants first, then weights in first-use order, all on
        # the sync ring (x/y traffic lives on gpsimd)
        maskadd_sb = wp.tile([T, T], BF16, tag="maskadd")
        nc.sync.dma_start(maskadd_sb[:], maskadd_d)
        ident_sb = wp.tile([128, 128], BF16, tag="ident")
        nc.sync.dma_start(ident_sb[:], ident_d)
        wq_sb = wload("wq", wq_d, KC, C, F8, nc.sync)
        wk_sb = wload("wk", wk_d, KC, C, F8, nc.sync)
        wv_sb = wload("wv", wv_d, KC, C, F8, nc.sync)
        wp_sb = wload("wp", wp_d, KC, C, F8, nc.sync)
        w1_sb = wload("w1", w1_d, KC, F, F8, nc.sync)
        w2_sb = wload("w2", w2_d, FM, C, F8, nc.sync)

        if has_b1:
            b1_sb = wp.tile([128, FM], F32, tag="b1")
            nc.sync.dma_start(b1_sb[:], b1_d.rearrange("(fm p) -> p fm", p=128))
        if has_bq:
            bq_sb = wp.tile([128, H], F32, tag="bq")
            nc.sync.dma_start(bq_sb[:], bq_d.rearrange("(h d) -> d h", d=128))
        if has_bk:
            bk_sb = wp.tile([128, H], F32, tag="bk")
            nc.sync.dma_start(bk_sb[:], bk_d.rearrange("(h d) -> d h", d=128))
        if has_bv:
            bv_sb = wp.tile([T, C], F32, tag="bv")
            nc.sync.dma_start(bv_sb[:], bv_d)
        if has_bp:
            bp_sb = wp.tile([128, C], F32, tag="bp")
            nc.sync.dma_start(bp_sb[:], bp_d)
        if has_b2:
            b2_sb = wp.tile([128, C], F32, tag="b2")
            nc.sync.dma_start(b2_sb[:], b2_d)

        # ---- LN helpers ----
        # Stats are collected per block into a shared per-pair agg tile;
        # the rsqrt (quake bit trick + 1 Newton step, all DVE, no ACT
        # table) runs ONCE per pair over both blocks' stats so its serial
        # chain amortizes.
        def ln_stats_collect(src, agg, j, pref):
            bst = st.tile([128, TCH, 6], F32, tag=pref + "bst")
            for i in range(TCH):
                nc.vector.bn_stats(bst[:, i, :], src[:, i, :])
                nc.vector.bn_aggr(agg[:, j, i, :], bst[:, i, :])

        def quake_pair(agg, pref):
            """agg [128, 2, TCH, 2] -> (rstd, nmr) [128, 2, TCH] f32."""
            mu = agg[:, :, :, 0]
            var = agg[:, :, :, 1]
            veps = st.tile([128, 2, TCH], F32, tag=pref + "veps")
            nc.vector.tensor_scalar_add(veps[:], var, EPS)
            ti = st.tile([128, 2, TCH], I32, tag=pref + "ti")
            nc.vector.tensor_scalar(
                out=ti[:], in0=veps[:].bitcast(I32), scalar1=1, scalar2=None,
                op0=OP.arith_shift_right)
            # magic - t == (t XOR -1) + (magic + 1)  (two's complement;
            # bitwise and arith ops can't share one instruction)
            nc.vector.tensor_scalar(
                out=ti[:], in0=ti[:], scalar1=-1, scalar2=None,
                op0=OP.bitwise_xor)
            nc.vector.tensor_scalar(
                out=ti[:], in0=ti[:], scalar1=QMAGIC + 1, scalar2=None,
                op0=OP.add)
            y0 = ti[:].bitcast(F32)
            a = st.tile([128, 2, TCH], F32, tag=pref + "nwa")
            nc.vector.tensor_mul(out=a[:], in0=y0, in1=y0)
            nc.vector.tensor_mul(out=a[:], in0=a[:], in1=veps[:])
            b = st.tile([128, 2, TCH], F32, tag=pref + "nwb")
            nc.vector.scalar_tensor_tensor(b[:], a[:], -0.5, y0,
                                           OP.mult, OP.mult)
            rstd = st.tile([128, 2, TCH], F32, tag=pref + "rstd")
            nc.vector.scalar_tensor_tensor(rstd[:], y0, 1.5, b[:],
                                           OP.mult, OP.add)
            nmr = st.tile([128, 2, TCH], F32, tag=pref + "nmr")
            nc.vector.scalar_tensor_tensor(nmr[:], mu, -1.0, rstd[:],
                                           OP.mult, OP.mult)
            return rstd, nmr

        def ln_apply(src, rstd, nmr, j, pref):
            # (x * rstd_p) + nmr_p as ONE DVE tensor_scalar (both scalars
            # are per-partition APs) - cheaper than the 720ns ACT op and
            # keeps the LN chain on a single engine.
            xn = ap_.tile([128, TCH, C], BF16, tag=pref + "xn", bufs=4)
            for i in range(TCH):
                nc.vector.tensor_scalar(
                    out=xn[:, i, :], in0=src[:, i, :],
                    scalar1=rstd[:, j, i : i + 1], scalar2=nmr[:, j, i : i + 1],
                    op0=OP.mult, op1=OP.add)
            return xn

        # ---- block stages ----
        def a1_pair(pr):
            """x loads, LN1 (pair-shared quake), PE transposes, fp8 cast."""
            out = []
            blks = (2 * pr, 2 * pr + 1)
            xsb = {}
            for blk in blks:
                row0 = blk * TOK
                x_sb = ap_.tile([128, TCH, C], BF16, tag="x", bufs=8)
                nc.gpsimd.dma_start(
                    x_sb[:],
                    x_d[row0 : row0 + TOK, :].rearrange("(ch p) c -> p ch c",
                                                        p=128))
                xsb[blk] = x_sb
            agg = st.tile([128, 2, TCH, 2], F32, tag="aagg")
            for j, blk in enumerate(blks):
                ln_stats_collect(xsb[blk], agg, j, "a")
            rstd, nmr = quake_pair(agg, "a")
            for j, blk in enumerate(blks):
                xn = ln_apply(xsb[blk], rstd, nmr, j, "a")
                xnT8 = ap_.tile([128, KC, TOK], F8, tag="axnT8", bufs=6)
                # transpose on the PE in two kc-halves, cast psum->fp8
                for half in range(2):
                    tp = ps.tile([128, 2 * TCH, 128], BF16, tag="pa", bufs=4)
                    for kk in range(2):
                        kc = 2 * half + kk
                        for mc in range(TCH):
                            nc.tensor.transpose(
                                tp[:, kk * TCH + mc, :],
                                xn[:, mc, kc * 128 : (kc + 1) * 128],
                                ident_sb[:])
                    nc.vector.tensor_copy(
                        out=xnT8[:, 2 * half : 2 * half + 2, :], in_=tp[:])
                out.append((xsb[blk], xnT8))
            return out

        def stage_a2(blk, xnT8):
            """QKV (fp8 DoubleRow) + masked scores + exp + softmax denom."""
            qt = ap_.tile([128, H, TOK], BF16, tag="qt")
            kt = ap_.tile([128, H, TOK], BF16, tag="kt")
            for dst, w_sb, bias_sb in ((qt, wq_sb, bq_sb if has_bq else None),
                                       (kt, wk_sb, bk_sb if has_bk else None)):
                for h in range(H):
                    p = ps.tile([128, 512], F32, tag="pa", bufs=4)
                    for kc in range(0, KC, 2):
                        nc.tensor.matmul(
                            p[:, 0:TOK],
                            w_sb[:, kc : kc + 2, h * 128 : (h + 1) * 128],
                            xnT8[:, kc : kc + 2, :],
                            start=(kc == 0), stop=(kc == KC - 2), perf_mode=DR)
                    if bias_sb is not None:
                        nc.scalar.activation(dst[:, h, :], p[:, 0:TOK],
                                             AF.Identity, scale=IS,
                                             bias=bias_sb[:, h : h + 1])
                    elif has_bq or has_bk:
                        # the IS^2 fold into exp_scale is off in bias mode
                        nc.scalar.activation(dst[:, h, :], p[:, 0:TOK],
                                             AF.Identity, scale=IS)
                    elif h % 2 == 0:
                        nc.scalar.activation(dst[:, h, :], p[:, 0:TOK],
                                             AF.Identity)
                    else:
                        nc.vector.tensor_copy(out=dst[:, h, :], in_=p[:, 0:TOK])

            # scores with additive causal mask preloaded into PSUM
            ee = ap_.tile([T, H * NB, T], BF16, tag="ee", bufs=4)
            dsum = st.tile([T, H * NB], F32, tag="dsum")
            for h in range(H):
                p = ps.tile([T, NB, T], F32, tag="pa", bufs=4)
                # keep each region's accumulation group consecutive: the
                # interleaved form (all starts, then all stops) computes
                # wrong results on hardware.
                for b in range(NB):
                    nc.tensor.matmul(p[:, b, :], ident_sb[:T, :T],
                                     maskadd_sb[:], start=True, stop=False)
                    nc.tensor.matmul(p[:, b, :], qt[:, h, b * T : (b + 1) * T],
                                     kt[:, h, b * T : (b + 1) * T],
                                     start=False, stop=True)
                sl = slice(h * NB, (h + 1) * NB)
                nc.scalar.activation(ee[:, sl, :], p[:], AF.Exp,
                                     scale=exp_scale)
            nc.vector.tensor_reduce(dsum[:], ee[:], axis=mybir.AxisListType.X,
                                    op=OP.add)
            # rr = IS / dsum  (folds away the x32 on wv; with bv the
            # descale already happened at the vt copy)
            rr = st.tile([T, H * NB], F32, tag="rr")
            if has_bv:
                nc.vector.reciprocal(rr[:], dsum[:])
            else:
                nc.vector.tensor_scalar_mul(rr[:], dsum[:], WS)
                nc.vector.reciprocal(rr[:], rr[:])
            nc.vector.tensor_mul(
                out=ee[:], in0=ee[:],
                in1=rr[:].unsqueeze(2).to_broadcast([T, H * NB, T]))

            # V projection (fp8 DoubleRow, per sequence)
            vt = ap_.tile([T, NB, C], BF16, tag="vt", bufs=4)
            for b in range(NB):
                p = ps.tile([T, 512], F32, tag="pa", bufs=4)
                for kc in range(0, KC, 2):
                    nc.tensor.matmul(p[:], xnT8[:, kc : kc + 2, b * T : (b + 1) * T],
                                     wv_sb[:, kc : kc + 2, :],
                                     start=(kc == 0), stop=(kc == KC - 2),
                                     perf_mode=DR)
                if has_bv:
                    nc.vector.scalar_tensor_tensor(vt[:, b, :], p[:], IS,
                                                   bv_sb[:], OP.mult, OP.add)
                else:
                    nc.scalar.activation(vt[:, b, :], p[:], AF.Identity)
            return vt, ee

        def stage_a2b(blk, vt, ee):
            """probs transpose + attn @ V -> ot8 (fp8, T-layout)."""
            pt = ee  # probs overwritten in place by their transpose
            for h in range(H):
                p = ps.tile([T, NB, T], BF16, tag="pa", bufs=4)
                for b in range(NB):
                    nc.tensor.transpose(p[:, b, :], ee[:, h * NB + b, :],
                                        ident_sb[:T, :T])
                nc.vector.tensor_copy(out=pt[:, h * NB : (h + 1) * NB, :], in_=p[:])
            ot8 = ap_.tile([128, H, TOK], F8, tag="ot8", bufs=4)
            for h in range(H):
                p = ps.tile([128, NB, T], F32, tag="pa", bufs=4)
                for b in range(NB):
                    nc.tensor.matmul(p[:, b, :], vt[:, b, h * 128 : (h + 1) * 128],
                                     pt[:, h * NB + b, :], start=True, stop=True)
                if h % 2 == 0:
                    nc.scalar.activation(ot8[:, h, :], p[:], AF.Identity)
                else:
                    nc.vector.tensor_copy(out=ot8[:, h, :], in_=p[:])
            return ot8

        def stage_b1(blk, x_sb, ot8, agg2, j):
            """proj + residual -> x2; LN2 stats into the pair agg tile."""
            x2 = ap_.tile([128, TCH, C], BF16, tag="x2", bufs=4)
            for mc in range(TCH):
                p = ps.tile([128, 512], F32, tag="pb", bufs=2)
                for kc in range(0, H, 2):
                    nc.tensor.matmul(p[:], ot8[:, kc : kc + 2, mc * 128 : (mc + 1) * 128],
                                     wp_sb[:, kc : kc + 2, :],
                                     start=(kc == 0), stop=(kc == H - 2),
                                     perf_mode=DR)
                if has_bp:
                    nc.vector.scalar_tensor_tensor(p[:], p[:], IS, bp_sb[:],
                                                   OP.mult, OP.add)
                    nc.vector.tensor_add(out=x2[:, mc, :], in0=p[:],
                                         in1=x_sb[:, mc, :])
                else:
                    # x2 = proj * IS + x
                    nc.vector.scalar_tensor_tensor(
                        x2[:, mc, :], p[:], IS, x_sb[:, mc, :], OP.mult, OP.add)
            ln_stats_collect(x2, agg2, j, "b")
            return x2

        def stage_b1b(blk, x2, rstd2, nmr2, j):
            """LN2 apply (ACT) -> xn2."""
            return ln_apply(x2, rstd2, nmr2, j, "b")

        def stage_b2(blk, xn2):
            """LN2 transpose on the PE + one batched fp8 cast."""
            tp = ps.tile([128, KC * TCH, 128], BF16, tag="pb", bufs=2)
            for kc in range(KC):
                for mc in range(TCH):
                    nc.tensor.transpose(tp[:, kc * TCH + mc, :],
                                        xn2[:, mc, kc * 128 : (kc + 1) * 128],
                                        ident_sb[:])
            xn2T8 = ap_.tile([128, KC, TOK], F8, tag="xn2T8")
            nc.vector.tensor_copy(out=xn2T8[:], in_=tp[:])
            return xn2T8

        def stage_b3(blk, x2, xn2T8):
            """MLP (fp8 DoubleRow) + residual + store."""
            row0 = blk * TOK
            ht = hp.tile([128, FM, TOK], F8, tag="ht")
            for fg in range(0, FM, 2):
                p = ps.tile([128, 2, 512], F32, tag="pb", bufs=2)
                for fi in range(2):
                    fm = fg + fi
                    for kc in range(0, KC, 2):
                        nc.tensor.matmul(
                            p[:, fi, 0:TOK],
                            w1_sb[:, kc : kc + 2, fm * 128 : (fm + 1) * 128],
                            xn2T8[:, kc : kc + 2, :],
                            start=(kc == 0), stop=(kc == KC - 2), perf_mode=DR)
                if has_b1:
                    for fi in range(2):
                        nc.scalar.activation(ht[:, fg + fi, :], p[:, fi, 0:TOK],
                                             AF.Gelu, scale=IS,
                                             bias=b1_sb[:, fg + fi : fg + fi + 1])
                else:
                    nc.scalar.activation(ht[:, fg : fg + 2, :], p[:, :, 0:TOK],
                                         AF.Gelu, scale=IS)

            xo = ap_.tile([128, TCH, C], F32, tag="xo")
            for mc in range(TCH):
                p = ps.tile([128, 512], F32, tag="pb", bufs=2)
                for fk in range(0, FM, 2):
                    nc.tensor.matmul(p[:], ht[:, fk : fk + 2, mc * 128 : (mc + 1) * 128],
                                     w2_sb[:, fk : fk + 2, :],
                                     start=(fk == 0), stop=(fk == FM - 2),
                                     perf_mode=DR)
                if has_b2:
                    nc.vector.scalar_tensor_tensor(p[:], p[:], IS, b2_sb[:],
                                                   OP.mult, OP.add)
                    nc.vector.tensor_add(out=xo[:, mc, :], in0=p[:],
                                         in1=x2[:, mc, :])
                else:
                    nc.vector.scalar_tensor_tensor(
                        xo[:, mc, :], p[:], IS, x2[:, mc, :], OP.mult, OP.add)
            nc.gpsimd.dma_start(
                y_d[row0 : row0 + TOK, :].rearrange("(ch p) c -> p ch c", p=128),
                xo[:])

        # ---- software-pipelined emission over block PAIRS ----
        # Per iteration (pair pr; prev pair P1,P2):
        #   a1_pair(pr+2)   LN1 chain, two pairs of latency slack
        #   b2(P1) b2(P2)   prev pair LN2 transposes (applies done last iter)
        #   a2(A) a2(B)     attention front (PE-heavy, hides LN2+softmax
        #   b3(P1) b3(P2)   prev pair MLPs    chains queued further down)
        #   a2b(A) a2b(B)   attention tail
        #   b1 b1 quake b1b b1b   proj + LN2 chain; latency hidden until
        #                         the NEXT iteration's b2 needs xn2
        # The ACT table flips exp_and_others <-> gelu_and_others only twice
        # per pair (all Identity/LN applies live in every set).
        assert nblk % 2 == 0
        npair = nblk // 2
        xs, xnTs, sm, ots, x2s, xn2s = {}, {}, {}, {}, {}, {}

        def do_a1(pr):
            for blk, (x_sb, xnT8) in zip((2 * pr, 2 * pr + 1), a1_pair(pr)):
                xs[blk], xnTs[blk] = x_sb, xnT8

        def do_b1(pr):
            agg2 = st.tile([128, 2, TCH, 2], F32, tag="bagg")
            for j, blk in enumerate((2 * pr, 2 * pr + 1)):
                x2s[blk] = stage_b1(blk, xs.pop(blk), ots.pop(blk), agg2, j)
            rstd2, nmr2 = quake_pair(agg2, "b")
            for j, blk in enumerate((2 * pr, 2 * pr + 1)):
                xn2s[blk] = stage_b1b(blk, x2s[blk], rstd2, nmr2, j)

        do_a1(0)
        if npair > 1:
            do_a1(1)
        for pr in range(npair):
            if pr > 0:
                for blk in (2 * pr - 2, 2 * pr - 1):
                    xn2s[blk] = stage_b2(blk, xn2s.pop(blk))
            for blk in (2 * pr, 2 * pr + 1):
                sm[blk] = stage_a2(blk, xnTs.pop(blk))
            if pr > 0:
                for blk in (2 * pr - 2, 2 * pr - 1):
                    stage_b3(blk, x2s.pop(blk), xn2s.pop(blk))
            for blk in (2 * pr, 2 * pr + 1):
                ots[blk] = stage_a2b(blk, *sm.pop(blk))
            do_b1(pr)
            # a1 is latency-tolerant (consumed two iterations later), so
            # its DVE/ACT/PE work goes at the queue TAILS: the in-order
            # engine FIFOs then serve this iteration's critical items
            # (casts, qt/kt copies, softmax) first.
            if pr + 2 < npair:
                do_a1(pr + 2)
        for blk in (nblk - 2, nblk - 1):
            xn2s[blk] = stage_b2(blk, xn2s.pop(blk))
        for blk in (nblk - 2, nblk - 1):
            stage_b3(blk, x2s.pop(blk), xn2s.pop(blk))

    nc.compile()
    return nc


def fold(inputs):
    """Host-side exact folding of LN affines and biases into weights."""
    f32 = np.float32
    f8 = ml_dtypes.float8_e4m3
    g1 = np.asarray(inputs["g1"], f32)
    be1 = np.asarray(inputs["be1"], f32)
    g2 = np.asarray(inputs["g2"], f32)
    be2 = np.asarray(inputs["be2"], f32)

    def headcat(w):  # [H, C, D] -> [C, H*D]
        return np.concatenate([w[h] for h in range(H)], axis=1)

    wq = headcat(np.asarray(inputs["wq"], f32))
    wk = headcat(np.asarray(inputs["wk"], f32))
    wv = headcat(np.asarray(inputs["wv"], f32))
    wp_ = np.asarray(inputs["w_proj"], f32)
    w1 = np.asarray(inputs["w1"], f32)
    w2 = np.asarray(inputs["w2"], f32)

    wq_f = g1[:, None] * wq
    wk_f = g1[:, None] * wk
    wv_f = g1[:, None] * wv
    bq = be1 @ wq
    bk = be1 @ wk
    bv = be1 @ wv
    bp = np.asarray(inputs["b_proj"], f32)
    w1_f = g2[:, None] * w1
    b1 = np.asarray(inputs["b1"], f32) + be2 @ w1
    b2 = np.asarray(inputs["b2"], f32)

    def q8(w):
        return np.clip(w * WS, -240.0, 240.0).astype(f8)

    # Masked positions must vanish under exp(exp_scale * psum); the psum
    # carries q.k at x1024 (both operands hold x32 weights), so the
    # additive mask must be large enough that exp_scale*mask << -40.
    maskadd = np.where(np.tril(np.ones((T, T), np.float32)) > 0, 0.0,
                       -1e6).astype(ml_dtypes.bfloat16)
    ident = np.eye(128, dtype=ml_dtypes.bfloat16)

    staged = {
        "wq": q8(wq_f),
        "wk": q8(wk_f),
        "wv": q8(wv_f),
        "wp": q8(wp_),
        "w1": q8(w1_f),
        "w2": q8(w2),
        "maskadd": maskadd,
        "ident": ident,
    }
    flags = {
        "has_bq": bool(np.any(bq)),
        "has_bk": bool(np.any(bk)),
        "has_bv": bool(np.any(bv)),
        "has_bp": bool(np.any(bp)),
        "has_b1": bool(np.any(b1)),
        "has_b2": bool(np.any(b2)),
    }
    if flags["has_b1"]:
        staged["b1"] = b1
    if flags["has_bq"]:
        staged["bq"] = bq
    if flags["has_bk"]:
        staged["bk"] = bk
    if flags["has_bv"]:
        staged["bv_b"] = np.broadcast_to(bv, (T, C)).copy()
    if flags["has_bp"]:
        staged["bp_b"] = np.broadcast_to(bp, (128, C)).copy()
    if flags["has_b2"]:
        staged["b2_b"] = np.broadcast_to(b2, (128, C)).copy()
    return staged, flags


_CACHE = {}


def kernel(**inputs):
    inputs = {k: np.asarray(v) for k, v in inputs.items()}
    staged, flags = fold(inputs)
    key = tuple(sorted(flags.items()))
    if key not in _CACHE:
        _CACHE[key] = build(**flags)
    nc = _CACHE[key]

    x = np.asarray(inputs["x"], np.float32).astype(ml_dtypes.bfloat16)
    x = x.reshape(B, T * C)
    in_maps = []
    for c in range(NCORES):
        m = dict(staged)
        m["x"] = x[c * SEQ_PER_CORE : (c + 1) * SEQ_PER_CORE].reshape(S, C)
        in_maps.append(m)

    res = bass_utils.run_bass_kernel_spmd(nc, in_maps, core_ids=list(range(NCORES)))
    out = np.concatenate([r["y"] for r in res.results], axis=0)
    return out.reshape(B, T, C).astype(np.float32)
